# revision 16
# baseline (speedup 1.0000x reference)
"""MoD-router FFN kernel for 8 TRN2 NeuronCores (self-contained).

Math note: the reference applies softmax over a size-1 axis, which yields
all-ones scores for ANY input; jax.lax.top_k is stable, so the selected
token indices are always [0..NUM_TOKENS) per batch row. The router weights
(Wp, bp) therefore cannot affect the output, and the kernel computes

    out = gelu_tanh(x[:, :2048, :] @ W1 + b1) @ W2 + b2

Sharding: data-parallel over the 4*2048 = 8192 selected token rows ->
1024 rows per core. Each core runs a fused transposed FFN in bf16
(rel err ~3e-3 vs the 2e-2 gate; PE rate is the same 1 cycle/row as
float32r but DMA traffic halves):
  H^T = gelu(W1^T @ X^T + b1)   (per F-block of 512, kept in SBUF bf16)
  out^T += W2_blk^T @ H^T_blk   (accumulated in SBUF fp32 via DVE)

Schedule (from baseline trace analysis):
  - DMA issues are ~650ns each and serialize per queue; spread them over
    the sync/act/vector queues so the first matmul starts ~10us earlier.
  - fb=0 FFN1 runs k-outer across 8 concurrent psum chains so compute
    starts as soon as xt[0] lands and is never DMA-paced.
  - Steady state is software-pipelined: window fb runs FFN1(fb) then
    FFN2(fb-1), so FFN2 never waits on the gelu of its own window.
  - out[d] DMAs are issued inside the last window right after each d's
    final accumulation, spread across queues (the baseline serialized
    them at the very end, adding ~9us of pure tail).
b2 is applied on the host (it is all-zeros in this problem's inputs).
"""

import numpy as np

B, S, D, F = 4, 4096, 2048, 8192
NUM_TOKENS = 2048
NCORES = 8
ROWS = (B * NUM_TOKENS) // NCORES     # 1024 rows per core
P = 128
KT = D // P                           # 16 k-subtiles over D (FFN1 contraction)
FT = F // P                           # 64 f-tiles
FB = 16                               # F-blocks of 512
FSUB = FT // FB                       # 4 f-subtiles per block
DT = D // P                           # 16 d-tiles
NCH = 2                               # row chunks of 512 (PSUM bank limit)
KS2 = 4                               # k-subtiles per F-block in FFN2

_CACHE = {}


def _build():
    import concourse.bass as bass
    import concourse.mybir as mybir
    import concourse.tile as tile
    from concourse import bacc

    f32 = mybir.dt.float32
    bf16 = mybir.dt.bfloat16
    GELU = mybir.ActivationFunctionType.Gelu_apprx_tanh

    nc = bacc.Bacc()
    # xt[k] : [P, ROWS] k-th 128-slice of X^T (bf16)
    xt = nc.declare_dram_parameter("xt", [KT, P, ROWS], bf16, isOutput=False)
    # w1[fb]: [P, KT*FSUB*P] with column order (k, fs, c) so a k-slice is
    # contiguous (fb=0 streams k-sliced for the warmup).
    w1 = nc.declare_dram_parameter("w1", [FB, P, KT * FSUB * P], bf16, isOutput=False)
    # w2[fb]: [P, DT*KS2*P] with column order (d, ks, c)
    w2 = nc.declare_dram_parameter("w2", [FB, P, DT * KS2 * P], bf16, isOutput=False)
    b1 = nc.declare_dram_parameter("b1", [P, FT], f32, isOutput=False)
    out = nc.declare_dram_parameter("out", [DT, P, ROWS], f32, isOutput=True)

    with tile.TileContext(nc) as tc:
        with (
            tc.tile_pool(name="resident", bufs=1) as res,
            tc.tile_pool(name="w1p", bufs=2) as w1p,
            tc.tile_pool(name="w2p", bufs=2) as w2p,
            tc.tile_pool(name="htp", bufs=8) as htp,
            tc.tile_pool(name="ps", bufs=8, space="PSUM") as ps,
        ):
            xt_sb = [res.tile([P, ROWS], bf16, name=f"xt{k}") for k in range(KT)]
            oacc = [res.tile([P, ROWS], f32, name=f"oacc{d}") for d in range(DT)]
            b1_sb = res.tile([P, FT], f32, name="b1sb")
            w1_sb = [None] * FB
            w2_sb = [None] * FB

            # PE pre-warm: the tensor engine clock ramps to full speed only
            # after ~3us of continuous execution. Burn ~3us on dummy matmuls
            # over a zeroed scratch tile while the first xt/w1 DMAs are
            # still in flight, so real matmuls start at full clock.
            scratch = res.tile([P, 512], bf16, name="prewarm_src")
            nc.vector.memset(scratch[:], 0.0)
            ps_warm = ps.tile([P, 512], f32, name="ps_prewarm", tag="ps")
            NWARM = 18  # ~5.4us: bridges until the xt stream is ahead
            for i in range(NWARM):
                nc.tensor.matmul(ps_warm[:], scratch[:, 0:P], scratch[:],
                                 start=(i == 0), stop=(i == NWARM - 1))

            # --- startup DMAs, spread across queues -----------------------
            # xt tiles stream on sync (~0.6us serialized issue each); first
            # 4 tiles split in halves so the n=0 warmup chains start on
            # half-arrival. The scalar queue streams w1[0] k-sliced (slice k
            # lands before the warmup's k-th step), w1[1] after slice 5.
            w1_sb[0] = w1p.tile([P, KT * FSUB * P], bf16, name="w1t0", tag="w1t")
            w1_sb[1] = w1p.tile([P, KT * FSUB * P], bf16, name="w1t1", tag="w1t")
            CSL = FSUB * P  # columns per k-slice
            for k in range(KT):
                if k < 4:
                    for h in range(2):
                        nc.sync.dma_start(out=xt_sb[k][:, h * 512:(h + 1) * 512],
                                          in_=xt[k, :, h * 512:(h + 1) * 512])
                else:
                    nc.sync.dma_start(out=xt_sb[k][:], in_=xt[k])
                nc.scalar.dma_start(out=w1_sb[0][:, k * CSL:(k + 1) * CSL],
                                    in_=w1[0, :, k * CSL:(k + 1) * CSL])
                # w1[1] after slice 13: late enough not to starve the
                # warmup's xt stream, early enough to land before FFN1(1)
                # starts (~32us).
                if k == 13:
                    nc.scalar.dma_start(out=w1_sb[1][:], in_=w1[1])
            # w2[0] at the end of the sync queue so its 2MB stays out of the
            # warmup's HBM window (deadline ~64us); b1 tiny, on gpsimd.
            nc.gpsimd.dma_start(out=b1_sb[:], in_=b1[:])
            w2_sb[0] = w2p.tile([P, DT * KS2 * P], bf16, name="w2t0", tag="w2t")
            nc.sync.dma_start(out=w2_sb[0][:], in_=w2[0])

            ht = {}  # (fb, fs) -> [P, ROWS] bf16 tile

            def ffn1_warmup():
                # k-outer across all 8 (fs, n) chains: compute starts once
                # xt[0] + w1[0]'s k=0 slice land, and consumes xt[k] at
                # ~1.7us/step vs ~0.8us/step DMA supply.
                chains = [(fs, n) for n in range(NCH) for fs in range(FSUB)]
                psums = {c: ps.tile([P, 512], f32, name=f"ps_w{c[0]}_{c[1]}", tag="ps")
                         for c in chains}
                for k in range(KT):
                    for fs, n in chains:
                        nc.tensor.matmul(
                            psums[(fs, n)][:],
                            w1_sb[0][:, (k * FSUB + fs) * P:(k * FSUB + fs + 1) * P],
                            xt_sb[k][:, n * 512:(n + 1) * 512],
                            start=(k == 0), stop=(k == KT - 1),
                        )
                for fs in range(FSUB):
                    h = htp.tile([P, ROWS], bf16, name=f"ht_0_{fs}", tag="ht")
                    ht[(0, fs)] = h
                    for n in range(NCH):
                        nc.scalar.activation(
                            h[:, n * 512:(n + 1) * 512], psums[(fs, n)][:],
                            GELU, bias=b1_sb[:, fs:fs + 1],
                        )

            def ffn1(fb):
                for fs in range(FSUB):
                    h = htp.tile([P, ROWS], bf16, name=f"ht_{fb}_{fs}", tag="ht")
                    ht[(fb, fs)] = h
                    for n in range(NCH):
                        psum = ps.tile([P, 512], f32, name=f"ps1_{fb}_{fs}_{n}", tag="ps")
                        for k in range(KT):
                            nc.tensor.matmul(
                                psum[:],
                                w1_sb[fb][:, (k * FSUB + fs) * P:(k * FSUB + fs + 1) * P],
                                xt_sb[k][:, n * 512:(n + 1) * 512],
                                start=(k == 0), stop=(k == KT - 1),
                            )
                        nc.scalar.activation(
                            h[:, n * 512:(n + 1) * 512], psum[:],
                            GELU, bias=b1_sb[:, fb * FSUB + fs:fb * FSUB + fs + 1],
                        )

            def ffn2(fb):
                last = fb == FB - 1
                for d in range(DT):
                    for n in range(NCH):
                        psum = ps.tile([P, 512], f32, name=f"ps2_{fb}_{d}_{n}", tag="ps")
                        for ks in range(KS2):
                            nc.tensor.matmul(
                                psum[:],
                                w2_sb[fb][:, (d * KS2 + ks) * P:(d * KS2 + ks + 1) * P],
                                ht[(fb, ks)][:, n * 512:(n + 1) * 512],
                                start=(ks == 0), stop=(ks == KS2 - 1),
                            )
                        dst = oacc[d][:, n * 512:(n + 1) * 512]
                        if fb == 0:
                            nc.vector.tensor_scalar_add(dst, psum[:], 0.0)
                        else:
                            nc.vector.tensor_add(dst, dst, psum[:])
                    if last:
                        # stream the finished d-tile out now, rotating queues
                        q = (nc.sync, nc.scalar, nc.gpsimd)[d % 3]
                        q.dma_start(out=out[d], in_=oacc[d][:])

            # --- pipelined schedule --------------------------------------
            # window fb: [prefetch w1(fb+1), w2(fb)] FFN1(fb) ; FFN2(fb-1)
            ffn1_warmup()
            for fb in range(1, FB):
                if fb + 1 < FB:
                    w1_sb[fb + 1] = w1p.tile([P, KT * FSUB * P], bf16,
                                             name=f"w1t{fb+1}", tag="w1t")
                    nc.scalar.dma_start(out=w1_sb[fb + 1][:], in_=w1[fb + 1])
                w2_sb[fb] = w2p.tile([P, DT * KS2 * P], bf16,
                                     name=f"w2t{fb}", tag="w2t")
                nc.sync.dma_start(out=w2_sb[fb][:], in_=w2[fb])
                ffn1(fb)
                ffn2(fb - 1)
            ffn2(FB - 1)

    nc.compile()
    return nc


def _get_nc():
    if "nc" not in _CACHE:
        _CACHE["nc"] = _build()
    return _CACHE["nc"]


def _prep_inputs(x, W1, b1):
    """Host-side shard + layout prep -> per-core in_maps."""
    import ml_dtypes

    bf = ml_dtypes.bfloat16
    xs = np.asarray(x, np.float32)[:, :NUM_TOKENS, :].reshape(B * NUM_TOKENS, D)
    # w1h[fb, p, (k, fs, c)] = W1[k*128+p, (fb*4+fs)*128+c]
    w1h = np.ascontiguousarray(
        np.asarray(W1, np.float32).reshape(KT, P, FB, FSUB, P)
        .transpose(2, 1, 0, 3, 4).reshape(FB, P, KT * FSUB * P)).astype(bf)
    b1h = np.ascontiguousarray(np.asarray(b1, np.float32).reshape(FT, P).T)
    in_maps = []
    for c in range(NCORES):
        xc = xs[c * ROWS:(c + 1) * ROWS]                        # [1024, 2048]
        xth = np.ascontiguousarray(xc.T.reshape(KT, P, ROWS)).astype(bf)
        in_maps.append({"xt": xth, "w1": w1h, "b1": b1h})
    return in_maps


def _prep_w2(W2):
    import ml_dtypes

    # w2h[fb, p, (d, ks, c)] = W2[(fb*4+ks)*128+p, d*128+c]
    return np.ascontiguousarray(
        np.asarray(W2, np.float32).reshape(FB, KS2, P, DT, P)
        .transpose(0, 2, 3, 1, 4).reshape(FB, P, DT * KS2 * P)
    ).astype(ml_dtypes.bfloat16)


def _gather(results, b2):
    out = np.empty((B * NUM_TOKENS, D), dtype=np.float32)
    for c in range(NCORES):
        oc = results[c]["out"]                                  # [d, p, n]
        out[c * ROWS:(c + 1) * ROWS] = np.asarray(oc, np.float32).reshape(D, ROWS).T
    b2 = np.asarray(b2, np.float32)
    if b2.any():
        out += b2
    return out.reshape(B, NUM_TOKENS, D)


def kernel(x, Wp, bp, W1, b1, W2, b2, **_unused):
    from concourse.bass_utils import run_bass_kernel_spmd

    in_maps = _prep_inputs(x, W1, b1)
    w2h = _prep_w2(W2)
    for m in in_maps:
        m["w2"] = w2h
    nc = _get_nc()
    res = run_bass_kernel_spmd(nc, in_maps, list(range(NCORES)))
    return _gather(res.results, b2)


# revision 18
# speedup vs baseline: 1.0055x; 1.0055x over previous
"""MoD-router FFN kernel for 8 TRN2 NeuronCores (self-contained).

Math note: the reference applies softmax over a size-1 axis, which yields
all-ones scores for ANY input; jax.lax.top_k is stable, so the selected
token indices are always [0..NUM_TOKENS) per batch row. The router weights
(Wp, bp) therefore cannot affect the output, and the kernel computes

    out = gelu_tanh(x[:, :2048, :] @ W1 + b1) @ W2 + b2

Sharding: data-parallel over the 4*2048 = 8192 selected token rows ->
1024 rows per core. Each core runs a fused transposed FFN in bf16
(rel err ~3e-3 vs the 2e-2 gate; PE rate is the same 1 cycle/row as
float32r but DMA traffic halves):
  H^T = gelu(W1^T @ X^T + b1)   (per F-block of 512, kept in SBUF bf16)
  out^T += W2_blk^T @ H^T_blk   (accumulated in SBUF fp32 via DVE)

Schedule (from baseline trace analysis):
  - DMA issues are ~650ns each and serialize per queue; spread them over
    the sync/act/vector queues so the first matmul starts ~10us earlier.
  - fb=0 FFN1 runs k-outer across 8 concurrent psum chains so compute
    starts as soon as xt[0] lands and is never DMA-paced.
  - Steady state is software-pipelined: window fb runs FFN1(fb) then
    FFN2(fb-1), so FFN2 never waits on the gelu of its own window.
  - out[d] DMAs are issued inside the last window right after each d's
    final accumulation, spread across queues (the baseline serialized
    them at the very end, adding ~9us of pure tail).
b2 is applied on the host (it is all-zeros in this problem's inputs).
"""

import numpy as np

B, S, D, F = 4, 4096, 2048, 8192
NUM_TOKENS = 2048
NCORES = 8
ROWS = (B * NUM_TOKENS) // NCORES     # 1024 rows per core
P = 128
KT = D // P                           # 16 k-subtiles over D (FFN1 contraction)
FT = F // P                           # 64 f-tiles
FB = 16                               # F-blocks of 512
FSUB = FT // FB                       # 4 f-subtiles per block
DT = D // P                           # 16 d-tiles
NCH = 2                               # row chunks of 512 (PSUM bank limit)
KS2 = 4                               # k-subtiles per F-block in FFN2

_CACHE = {}


def _build():
    import concourse.bass as bass
    import concourse.mybir as mybir
    import concourse.tile as tile
    from concourse import bacc

    f32 = mybir.dt.float32
    bf16 = mybir.dt.bfloat16
    GELU = mybir.ActivationFunctionType.Gelu_apprx_tanh

    nc = bacc.Bacc()
    # xt[k] : [P, ROWS] k-th 128-slice of X^T (bf16)
    xt = nc.declare_dram_parameter("xt", [KT, P, ROWS], bf16, isOutput=False)
    # w1[fb]: [P, KT*FSUB*P] with column order (k, fs, c) so a k-slice is
    # contiguous (fb=0 streams k-sliced for the warmup).
    w1 = nc.declare_dram_parameter("w1", [FB, P, KT * FSUB * P], bf16, isOutput=False)
    # w2[fb]: [P, DT*KS2*P] with column order (d, ks, c)
    w2 = nc.declare_dram_parameter("w2", [FB, P, DT * KS2 * P], bf16, isOutput=False)
    b1 = nc.declare_dram_parameter("b1", [P, FT], f32, isOutput=False)
    out = nc.declare_dram_parameter("out", [DT, P, ROWS], f32, isOutput=True)

    with tile.TileContext(nc) as tc:
        with (
            tc.tile_pool(name="resident", bufs=1) as res,
            tc.tile_pool(name="w1p", bufs=2) as w1p,
            tc.tile_pool(name="w2p", bufs=2) as w2p,
            tc.tile_pool(name="htp", bufs=8) as htp,
            tc.tile_pool(name="ps", bufs=8, space="PSUM") as ps,
        ):
            xt_sb = [res.tile([P, ROWS], bf16, name=f"xt{k}") for k in range(KT)]
            oacc = [res.tile([P, ROWS], f32, name=f"oacc{d}") for d in range(DT)]
            b1_sb = res.tile([P, FT], f32, name="b1sb")
            w1_sb = [None] * FB
            w2_sb = [None] * FB

            # PE pre-warm: the tensor engine clock ramps to full speed only
            # after ~3us of continuous execution. Burn ~3us on dummy matmuls
            # over a zeroed scratch tile while the first xt/w1 DMAs are
            # still in flight, so real matmuls start at full clock.
            scratch = res.tile([P, 512], bf16, name="prewarm_src")
            nc.vector.memset(scratch[:], 0.0)
            ps_warm = ps.tile([P, 512], f32, name="ps_prewarm", tag="ps")
            NWARM = 18  # ~5.4us: bridges until the xt stream is ahead
            for i in range(NWARM):
                nc.tensor.matmul(ps_warm[:], scratch[:, 0:P], scratch[:],
                                 start=(i == 0), stop=(i == NWARM - 1))

            # --- startup DMAs, spread across queues -----------------------
            # xt tiles stream on sync (~0.6us serialized issue each); first
            # 4 tiles split in halves so the n=0 warmup chains start on
            # half-arrival. The scalar queue streams w1[0] k-sliced (slice k
            # lands before the warmup's k-th step), then w1[1].
            w1_sb[0] = w1p.tile([P, KT * FSUB * P], bf16, name="w1t0", tag="w1t")
            w1_sb[1] = w1p.tile([P, KT * FSUB * P], bf16, name="w1t1", tag="w1t")
            CSL = FSUB * P  # columns per k-slice
            for k in range(KT):
                if k < 4:
                    for h in range(2):
                        nc.sync.dma_start(out=xt_sb[k][:, h * 512:(h + 1) * 512],
                                          in_=xt[k, :, h * 512:(h + 1) * 512])
                else:
                    nc.sync.dma_start(out=xt_sb[k][:], in_=xt[k])
                nc.scalar.dma_start(out=w1_sb[0][:, k * CSL:(k + 1) * CSL],
                                    in_=w1[0, :, k * CSL:(k + 1) * CSL])
            # w1[1] after the slices (issuing it earlier starves the
            # warmup's xt stream of HBM bandwidth — measured); w2[0] at the
            # end of the sync queue so its 2MB stays out of the warmup's
            # HBM window (deadline ~64us); b1 tiny, on gpsimd.
            nc.scalar.dma_start(out=w1_sb[1][:], in_=w1[1])
            nc.gpsimd.dma_start(out=b1_sb[:], in_=b1[:])
            w2_sb[0] = w2p.tile([P, DT * KS2 * P], bf16, name="w2t0", tag="w2t")
            nc.sync.dma_start(out=w2_sb[0][:], in_=w2[0])

            ht = {}  # (fb, fs) -> [P, ROWS] bf16 tile

            def ffn1_warmup():
                # k-outer across all 8 (fs, n) chains: compute starts once
                # xt[0] + w1[0]'s k=0 slice land, and consumes xt[k] at
                # ~1.7us/step vs ~0.8us/step DMA supply.
                chains = [(fs, n) for n in range(NCH) for fs in range(FSUB)]
                psums = {c: ps.tile([P, 512], f32, name=f"ps_w{c[0]}_{c[1]}", tag="ps")
                         for c in chains}
                for k in range(KT):
                    for fs, n in chains:
                        nc.tensor.matmul(
                            psums[(fs, n)][:],
                            w1_sb[0][:, (k * FSUB + fs) * P:(k * FSUB + fs + 1) * P],
                            xt_sb[k][:, n * 512:(n + 1) * 512],
                            start=(k == 0), stop=(k == KT - 1),
                        )
                for fs in range(FSUB):
                    h = htp.tile([P, ROWS], bf16, name=f"ht_0_{fs}", tag="ht")
                    ht[(0, fs)] = h
                    for n in range(NCH):
                        nc.scalar.activation(
                            h[:, n * 512:(n + 1) * 512], psums[(fs, n)][:],
                            GELU, bias=b1_sb[:, fs:fs + 1],
                        )

            def ffn1(fb):
                for fs in range(FSUB):
                    h = htp.tile([P, ROWS], bf16, name=f"ht_{fb}_{fs}", tag="ht")
                    ht[(fb, fs)] = h
                    for n in range(NCH):
                        psum = ps.tile([P, 512], f32, name=f"ps1_{fb}_{fs}_{n}", tag="ps")
                        for k in range(KT):
                            nc.tensor.matmul(
                                psum[:],
                                w1_sb[fb][:, (k * FSUB + fs) * P:(k * FSUB + fs + 1) * P],
                                xt_sb[k][:, n * 512:(n + 1) * 512],
                                start=(k == 0), stop=(k == KT - 1),
                            )
                        nc.scalar.activation(
                            h[:, n * 512:(n + 1) * 512], psum[:],
                            GELU, bias=b1_sb[:, fb * FSUB + fs:fb * FSUB + fs + 1],
                        )

            def ffn2(fb):
                last = fb == FB - 1
                for d in range(DT):
                    for n in range(NCH):
                        psum = ps.tile([P, 512], f32, name=f"ps2_{fb}_{d}_{n}", tag="ps")
                        for ks in range(KS2):
                            nc.tensor.matmul(
                                psum[:],
                                w2_sb[fb][:, (d * KS2 + ks) * P:(d * KS2 + ks + 1) * P],
                                ht[(fb, ks)][:, n * 512:(n + 1) * 512],
                                start=(ks == 0), stop=(ks == KS2 - 1),
                            )
                        dst = oacc[d][:, n * 512:(n + 1) * 512]
                        if fb == 0:
                            nc.vector.tensor_scalar_add(dst, psum[:], 0.0)
                        else:
                            nc.vector.tensor_add(dst, dst, psum[:])
                    if last:
                        # stream the finished d-tile out now, rotating queues
                        q = (nc.sync, nc.scalar, nc.gpsimd)[d % 3]
                        q.dma_start(out=out[d], in_=oacc[d][:])

            # --- pipelined schedule --------------------------------------
            # window fb: [prefetch w1(fb+1), w2(fb)] FFN1(fb) ; FFN2(fb-1)
            ffn1_warmup()
            for fb in range(1, FB):
                if fb + 1 < FB:
                    w1_sb[fb + 1] = w1p.tile([P, KT * FSUB * P], bf16,
                                             name=f"w1t{fb+1}", tag="w1t")
                    nc.scalar.dma_start(out=w1_sb[fb + 1][:], in_=w1[fb + 1])
                w2_sb[fb] = w2p.tile([P, DT * KS2 * P], bf16,
                                     name=f"w2t{fb}", tag="w2t")
                nc.sync.dma_start(out=w2_sb[fb][:], in_=w2[fb])
                ffn1(fb)
                ffn2(fb - 1)
            ffn2(FB - 1)

    nc.compile()
    return nc


def _get_nc():
    if "nc" not in _CACHE:
        _CACHE["nc"] = _build()
    return _CACHE["nc"]


def _prep_inputs(x, W1, b1):
    """Host-side shard + layout prep -> per-core in_maps."""
    import ml_dtypes

    bf = ml_dtypes.bfloat16
    xs = np.asarray(x, np.float32)[:, :NUM_TOKENS, :].reshape(B * NUM_TOKENS, D)
    # w1h[fb, p, (k, fs, c)] = W1[k*128+p, (fb*4+fs)*128+c]
    w1h = np.ascontiguousarray(
        np.asarray(W1, np.float32).reshape(KT, P, FB, FSUB, P)
        .transpose(2, 1, 0, 3, 4).reshape(FB, P, KT * FSUB * P)).astype(bf)
    b1h = np.ascontiguousarray(np.asarray(b1, np.float32).reshape(FT, P).T)
    in_maps = []
    for c in range(NCORES):
        xc = xs[c * ROWS:(c + 1) * ROWS]                        # [1024, 2048]
        xth = np.ascontiguousarray(xc.T.reshape(KT, P, ROWS)).astype(bf)
        in_maps.append({"xt": xth, "w1": w1h, "b1": b1h})
    return in_maps


def _prep_w2(W2):
    import ml_dtypes

    # w2h[fb, p, (d, ks, c)] = W2[(fb*4+ks)*128+p, d*128+c]
    return np.ascontiguousarray(
        np.asarray(W2, np.float32).reshape(FB, KS2, P, DT, P)
        .transpose(0, 2, 3, 1, 4).reshape(FB, P, DT * KS2 * P)
    ).astype(ml_dtypes.bfloat16)


def _gather(results, b2):
    out = np.empty((B * NUM_TOKENS, D), dtype=np.float32)
    for c in range(NCORES):
        oc = results[c]["out"]                                  # [d, p, n]
        out[c * ROWS:(c + 1) * ROWS] = np.asarray(oc, np.float32).reshape(D, ROWS).T
    b2 = np.asarray(b2, np.float32)
    if b2.any():
        out += b2
    return out.reshape(B, NUM_TOKENS, D)


def kernel(x, Wp, bp, W1, b1, W2, b2, **_unused):
    from concourse.bass_utils import run_bass_kernel_spmd

    in_maps = _prep_inputs(x, W1, b1)
    w2h = _prep_w2(W2)
    for m in in_maps:
        m["w2"] = w2h
    nc = _get_nc()
    res = run_bass_kernel_spmd(nc, in_maps, list(range(NCORES)))
    return _gather(res.results, b2)
